# revision 1
# baseline (speedup 1.0000x reference)
"""Trainium2 Bass kernel for nn_DCConv3dKernelPolynomials.

Computes out[m,n,b,p] = sum_k coeff[m,n,k] * psi_k(position[b,p,:])
where psi_k are the 23 real hydrogen-like wavefunctions (n<=4, l<=2).

Key math: with u=x/r, v=y/r, w=z/r, the angular factors are pure
polynomials (st^|m| e^{i|m|phi} = (u+iv)^|m|, P_l^m = st^|m| * poly(ct)),
so the device only needs sqrt/reciprocal/exp plus polynomial arithmetic.

Sharding: batch b -> core b (8 cores, 4096 points each). Per core:
  poly bf16 hi/lo [69, 4096] basis matrix (pointwise + PE transposes)
  out [4096(mn), 4096(pts)] = coeffT.T @ poly  (bf16 matmuls; K=69 packs
  the c_hi*p_hi + c_hi*p_lo + c_lo*p_hi fp32-recovery products into one
  single-pass matmul) streamed to HBM as 4 MiB DMAs (1 MiB pieces at the
  edges) on alternating HWDGE rings, with a triple-buffered stage.
"""

import math

import numpy as np

B = 8
PTS = 4096            # points per core
OUTC = INC = 64
MN = OUTC * INC       # 4096
NB = 23               # basis functions
NS = 3 * NB           # 69 = hi/lo/hi sections for fp32-ish matmul
NCORES = 8
PCHUNK = 32           # free-dim columns per partition in pointwise layout
NMT = MN // 128       # 32 mn tiles
NNT = PTS // 512      # 8 point tiles per mn tile


def _combos():
    combos = []
    for n in range(1, 5):
        for k in range(3):
            for m in range(-3, 4):
                if abs(m) <= k and k < n:
                    combos.append((n, k, m))
    return combos


COMBOS = _combos()
assert len(COMBOS) == NB


def _basis_scales():
    """Per-basis constant c_k so that psi_k = c_k * Rb_dev(n,l) * A_dev(l,m).

    Rb_dev / A_dev are the *unnormalized* tile products computed on device:
      Rb_dev(1,0)=e1, Rb_dev(2,0)=(2-r)e2, Rb_dev(2,1)=r*e2,
      Rb_dev(3,0)=(2r-(2/9)r^2-3)e3   [= -L_2^1(2r/3)e3]
      Rb_dev(3,1)=((8/3)r-(4/9)r^2)e3, Rb_dev(3,2)=r^2*e3,
      Rb_dev(4,0)=(4-3r+r^2/2-r^3/48)e4,
      Rb_dev(4,1)=(5r-(5/4)r^2+r^3/16)e4, Rb_dev(4,2)=(6-r/2)r^2*e4
      A_dev: 1, w, u, v, 3w^2-1, wu, wv, u^2-v^2, uv
    """
    fourpi = 4.0 * math.pi
    K00 = math.sqrt(1.0 / fourpi)
    K10 = math.sqrt(3.0 / fourpi)
    K11 = math.sqrt(3.0 / (2.0 * fourpi))
    K20 = math.sqrt(5.0 / fourpi)
    K21 = math.sqrt(5.0 / (6.0 * fourpi))
    K22 = math.sqrt(5.0 / (24.0 * fourpi))
    s2 = math.sqrt(2.0)

    def norm_r(n, l):
        return math.sqrt(
            (2.0 / n) ** 3 * math.factorial(n - l - 1)
            / (2 * n * math.factorial(n + l))
        )

    f = {
        (1, 0): norm_r(1, 0),
        (2, 0): norm_r(2, 0),
        (2, 1): norm_r(2, 1),
        (3, 0): -norm_r(3, 0),
        (3, 1): norm_r(3, 1),
        (3, 2): norm_r(3, 2) * (4.0 / 9.0),
        (4, 0): norm_r(4, 0),
        (4, 1): norm_r(4, 1),
        (4, 2): norm_r(4, 2) * 0.25,
    }
    a = {
        (0, 0): K00,
        (1, 0): K10,
        (1, 1): -s2 * K11,
        (1, -1): -s2 * K11,
        (2, 0): 0.5 * K20,
        (2, 1): -3.0 * s2 * K21,
        (2, -1): -3.0 * s2 * K21,
        (2, 2): 3.0 * s2 * K22,
        (2, -2): 6.0 * s2 * K22,
    }
    return np.array(
        [f[(n, l)] * a[(l, m)] for (n, l, m) in COMBOS], dtype=np.float64
    )


def poly_host(position):
    """Numpy replica of the device basis recipe (for self-checking)."""
    pos = np.asarray(position, dtype=np.float32)
    x, y, z = pos[..., 0], pos[..., 1], pos[..., 2]
    r2 = x * x + y * y + z * z
    r = np.sqrt(r2)
    ir = 1.0 / r
    u, v, w = x * ir, y * ir, z * ir
    e1, e2, e3, e4 = np.exp(-r), np.exp(-r / 2), np.exp(-r / 3), np.exp(-r / 4)
    rr = r * r
    A = {
        (0, 0): np.ones_like(r),
        (1, 0): w, (1, 1): u, (1, -1): v,
        (2, 0): 3 * w * w - 1, (2, 1): w * u, (2, -1): w * v,
        (2, 2): u * u - v * v, (2, -2): u * v,
    }
    Rb = {
        (1, 0): e1,
        (2, 0): (2 - r) * e2,
        (2, 1): r * e2,
        (3, 0): (2 * r - (2.0 / 9.0) * rr - 3) * e3,
        (3, 1): ((8.0 / 3.0) * r - (4.0 / 9.0) * rr) * e3,
        (3, 2): rr * e3,
        (4, 0): (4 - 3 * r + rr / 2 - rr * r / 48) * e4,
        (4, 1): (5 * r - 1.25 * rr + rr * r / 16) * e4,
        (4, 2): (6 - r / 2) * rr * e4,
    }
    c = _basis_scales()
    return np.stack(
        [
            (c[k] * Rb[(n, l)] * A[(l, m)]).astype(np.float32)
            for k, (n, l, m) in enumerate(COMBOS)
        ],
        axis=-1,
    )


_PROGRAM = None


def _build_program():
    import concourse.bacc as bacc
    import concourse.tile as tile
    from concourse import mybir
    from concourse.bass import ts
    from concourse.masks import make_identity

    f32 = mybir.dt.float32
    bf16 = mybir.dt.bfloat16
    AF = mybir.ActivationFunctionType
    ALU = mybir.AluOpType

    nc = bacc.Bacc(trn_type="TRN2")
    pos_d = nc.dram_tensor("position", [128, 96], f32, kind="ExternalInput")
    coefft_d = nc.dram_tensor("coefft", [NS, MN], bf16, kind="ExternalInput")
    out_d = nc.dram_tensor("out", [MN, PTS], f32, kind="ExternalOutput")

    with tile.TileContext(nc) as tc:
        with (
            tc.tile_pool(name="const", bufs=1) as const,
            tc.tile_pool(name="pw", bufs=1) as pw,
            tc.tile_pool(name="stage", bufs=3) as stage_pool,
            tc.tile_pool(name="psum_tr", bufs=2, space="PSUM") as psum_tr,
            tc.tile_pool(name="psum_mm", bufs=6, space="PSUM") as psum_mm,
        ):
            # inputs first: xyz gates the whole pointwise phase.
            # SWDGE (gpsimd) sprays across all 16 SDMA engines; the HWDGE
            # path put these small transfers on a single engine (~15us).
            xyz = const.tile([128, 96], f32, tag="xyz", name="xyz")
            nc.gpsimd.dma_start(out=xyz[:], in_=pos_d[:, :])
            coefft = const.tile([NS, MN], bf16, tag="coefft", name="coefft_sb")
            nc.gpsimd.dma_start(out=coefft[:], in_=coefft_d[:, :])

            ident = const.tile([128, 128], bf16, tag="ident", name="ident")
            make_identity(nc, ident[:])

            xyz3 = xyz[:].rearrange("p (c t) -> p c t", t=3)
            x, y, z = xyz3[:, :, 0], xyz3[:, :, 1], xyz3[:, :, 2]

            def T(tag):
                return pw.tile([128, PCHUNK], f32, tag=tag, name=tag)[:]

            def bcast3(ap2d, n):
                import concourse.bass as bass
                return bass.AP(
                    tensor=ap2d.tensor,
                    offset=ap2d.offset,
                    ap=[ap2d.ap[0], [0, n], ap2d.ap[1]],
                )

            # ---- pointwise: r, 1/r via exp(+-0.5*ln(r2)) -- one ACT table set
            r2, r, ir, rr = (T(t) for t in "r2 r ir rr".split())
            lnr2 = T("lnr2")
            nc.vector.tensor_mul(r2, x, x)
            tA, tB = T("tA"), T("tB")
            nc.vector.tensor_mul(tA, y, y)
            nc.vector.tensor_add(r2, r2, tA)
            nc.vector.tensor_mul(tB, z, z)
            nc.vector.tensor_add(r2, r2, tB)
            nc.scalar.activation(lnr2, r2, AF.Ln)
            nc.scalar.activation(r, lnr2, AF.Exp, scale=0.5)
            nc.scalar.activation(ir, lnr2, AF.Exp, scale=-0.5)

            # vwu[:, s, :] = (v, w, u); ang5[:, s, :] = (uv, wv, 3w^2-1, wu, u^2-v^2)
            vwu = pw.tile([128, 3, PCHUNK], f32, tag="vwu", name="vwu")[:]
            ang5 = pw.tile([128, 5, PCHUNK], f32, tag="ang5", name="ang5")[:]
            v, w, u = vwu[:, 0, :], vwu[:, 1, :], vwu[:, 2, :]
            uv, wv, a20, wu, a22 = (ang5[:, i, :] for i in range(5))
            nc.vector.tensor_mul(v, y, ir)
            nc.vector.tensor_mul(w, z, ir)
            nc.vector.tensor_mul(u, x, ir)
            nc.vector.tensor_mul(rr, r, r)

            e2, e3, e4 = T("e2"), T("e3"), T("e4")
            nc.scalar.activation(e2, r, AF.Exp, scale=-0.5)
            nc.scalar.activation(e3, r, AF.Exp, scale=-1.0 / 3.0)
            nc.scalar.activation(e4, r, AF.Exp, scale=-0.25)

            uu, vv = T("uu"), T("vv")
            nc.vector.tensor_mul(a20, w, w)
            nc.vector.tensor_scalar(a20, a20, 3.0, -1.0, ALU.mult, ALU.add)
            nc.vector.tensor_mul(uu, u, u)
            nc.vector.tensor_mul(vv, v, v)
            nc.vector.tensor_sub(a22, uu, vv)
            nc.vector.tensor_mul(uv, u, v)
            nc.vector.tensor_mul(wu, w, u)
            nc.vector.tensor_mul(wv, w, v)

            # ---- basis values into poly_s[:, c, k] (pre-transpose layout) ----
            poly_s = const.tile([128, PCHUNK, NB], f32, tag="poly_s", name="poly_s")
            slot = [poly_s[:, :, k] for k in range(NB)]

            poly_kc = poly_s[:].rearrange("p c k -> p k c")

            nc.scalar.activation(slot[0], r, AF.Exp, scale=-1.0)
            t20 = T("t20")
            nc.vector.tensor_scalar(t20, r, -1.0, 2.0, ALU.mult, ALU.add)
            nc.vector.tensor_mul(slot[1], t20, e2)
            rb21 = T("rb21")
            nc.vector.tensor_mul(rb21, r, e2)
            nc.vector.tensor_mul(poly_kc[:, 2:5, :], bcast3(rb21, 3), vwu)
            p30 = T("p30")
            nc.vector.tensor_scalar(p30, rr, 2.0 / 9.0, 3.0, ALU.mult, ALU.add)
            nc.vector.scalar_tensor_tensor(
                p30, r, 2.0, p30, ALU.mult, ALU.subtract
            )
            nc.vector.tensor_mul(slot[5], p30, e3)
            rb31 = T("rb31")
            nc.vector.tensor_scalar(
                rb31, r, -4.0 / 9.0, 8.0 / 3.0, ALU.mult, ALU.add
            )
            nc.vector.tensor_mul(rb31, rb31, r)
            nc.vector.tensor_mul(rb31, rb31, e3)
            nc.vector.tensor_mul(poly_kc[:, 6:9, :], bcast3(rb31, 3), vwu)
            rb32 = T("rb32")
            nc.vector.tensor_mul(rb32, rr, e3)
            nc.vector.tensor_mul(poly_kc[:, 9:14, :], bcast3(rb32, 5), ang5)
            p40, p40b = T("p40"), T("p40b")
            nc.vector.tensor_scalar(p40, r, -1.0 / 48.0, 0.5, ALU.mult, ALU.add)
            nc.vector.tensor_mul(p40, p40, rr)
            nc.vector.tensor_scalar(p40b, r, -3.0, 4.0, ALU.mult, ALU.add)
            nc.vector.tensor_add(p40, p40, p40b)
            nc.vector.tensor_mul(slot[14], p40, e4)
            rb41 = T("rb41")
            nc.vector.tensor_scalar(rb41, r, 1.0 / 16.0, -1.25, ALU.mult, ALU.add)
            nc.vector.tensor_mul(rb41, rb41, r)
            nc.vector.tensor_scalar(rb41, rb41, 5.0, None, ALU.add)
            nc.vector.tensor_mul(rb41, rb41, r)
            nc.vector.tensor_mul(rb41, rb41, e4)
            nc.vector.tensor_mul(poly_kc[:, 15:18, :], bcast3(rb41, 3), vwu)
            rb42 = T("rb42")
            nc.vector.tensor_scalar(rb42, r, -0.5, 6.0, ALU.mult, ALU.add)
            nc.vector.tensor_mul(rb42, rb42, rr)
            nc.vector.tensor_mul(rb42, rb42, e4)
            nc.vector.tensor_mul(poly_kc[:, 18:23, :], bcast3(rb42, 5), ang5)

            # ---- hi/lo bf16 split: hl[:, c, sec, k]; sec 0=hi 1=lo 2=hi ----
            hl = const.tile([128, PCHUNK, 3, NB], bf16, tag="hl", name="hl")
            nc.vector.tensor_copy(hl[:, :, 0, :], poly_s[:, :, :])
            nc.vector.tensor_sub(hl[:, :, 1, :], poly_s[:, :, :], hl[:, :, 0, :])
            nc.scalar.copy(hl[:, :, 2, :], hl[:, :, 0, :])

            # ---- transpose to poly_t [69, 4096] bf16 (point q = 32*p + c) ----
            poly_t = const.tile([NS, PTS], bf16, tag="poly_t", name="poly_t_sb")
            poly_t_v = poly_t[:].rearrange("k (p c) -> k p c", c=PCHUNK)
            for c in range(PCHUNK):
                pst = psum_tr.tile([NS, 128], bf16, tag="pst", name="pst")
                nc.tensor.transpose(pst[:], hl[:, c, :, :], ident[:])
                nc.vector.tensor_copy(poly_t_v[:, :, c], pst[:])

            # ---- matmuls + copies + output DMA ----
            # mt 0,1 and 30,31: 1 MiB piece-DMAs (fast start / short tail);
            # middle 28 tiles: pairs -> 14 stages with one 4 MiB DMA each.
            def mm_into(sl, mt, nt):
                ps = psum_mm.tile([128, 512], f32, tag="mmps", name="mmps")
                nc.tensor.matmul(
                    ps[:],
                    lhsT=coefft[:, ts(mt, 128)],
                    rhs=poly_t[:, ts(nt, 512)],
                    start=True,
                    stop=True,
                )
                if nt % 2 == 0:
                    nc.vector.tensor_copy(sl(ts(nt, 512)), ps[:])
                else:
                    nc.scalar.copy(sl(ts(nt, 512)), ps[:])

            edge = [0, 1, NMT - 2, NMT - 1]
            for mt in edge[:2]:
                stage = stage_pool.tile([128, PTS], f32, tag="stage_e", name="stage_e")
                for nt in range(NNT):
                    mm_into(lambda s: stage[:, s], mt, nt)
                    if nt % 2 == 1:
                        h = nt // 2
                        nc.sync.dma_start(
                            out=out_d[ts(mt, 128), ts(h, 1024)],
                            in_=stage[:, ts(h, 1024)],
                        )
            for mt2 in range(1, NMT // 2 - 1):
                stage = stage_pool.tile(
                    [128, 2, PTS], f32, tag="stage", name="stage"
                )
                for q in range(2):
                    mt = 2 * mt2 + q
                    for nt in range(NNT):
                        mm_into(lambda s: stage[:, q, s], mt, nt)
                dma_eng = nc.sync if mt2 % 2 == 0 else nc.scalar
                dest = out_d[2 * mt2 * 128:(2 * mt2 + 2) * 128, :].rearrange(
                    "(q p) j -> p q j", p=128
                )
                dma_eng.dma_start(out=dest, in_=stage[:, :, :])
            for mt in edge[2:]:
                stage = stage_pool.tile([128, PTS], f32, tag="stage_e", name="stage_e")
                for nt in range(NNT):
                    mm_into(lambda s: stage[:, s], mt, nt)
                    if nt % 2 == 1:
                        h = nt // 2
                        nc.scalar.dma_start(
                            out=out_d[ts(mt, 128), ts(h, 1024)],
                            in_=stage[:, ts(h, 1024)],
                        )

    nc.finalize()
    return nc


def _get_program():
    global _PROGRAM
    if _PROGRAM is None:
        _PROGRAM = _build_program()
    return _PROGRAM


def _prep_inputs(position, coefficients):
    import ml_dtypes

    pos = np.ascontiguousarray(np.asarray(position, dtype=np.float32))
    coeff = np.asarray(coefficients, dtype=np.float32)
    assert pos.shape == (B, PTS, 3) and coeff.shape == (OUTC, INC, NB)
    c = _basis_scales().astype(np.float32)
    C = (coeff * c).reshape(MN, NB).T.astype(np.float32)  # [23, 4096]
    c_hi = C.astype(ml_dtypes.bfloat16)
    c_lo = (C - c_hi.astype(np.float32)).astype(ml_dtypes.bfloat16)
    coefft = np.ascontiguousarray(np.concatenate([c_hi, c_hi, c_lo], axis=0))
    return [
        {"position": pos[b].reshape(128, 96), "coefft": coefft} for b in range(B)
    ]


def _assemble(results):
    return np.stack(
        [np.asarray(r["out"]).reshape(OUTC, INC, PTS) for r in results], axis=2
    )


def kernel(position, coefficients):
    from concourse import bass_utils

    nc = _get_program()
    in_maps = _prep_inputs(position, coefficients)
    res = bass_utils.run_bass_kernel_spmd(nc, in_maps, core_ids=list(range(NCORES)))
    return _assemble(res.results)


def kernel_traced(position, coefficients, trace_cores=None):
    """Like kernel() but captures an NTFF trace; returns (out, results)."""
    from concourse import bass_utils

    nc = _get_program()
    in_maps = _prep_inputs(position, coefficients)
    res = bass_utils.run_bass_kernel_spmd(
        nc,
        in_maps,
        core_ids=list(range(NCORES)),
        trace=True,
        trace_cores=trace_cores,
    )
    return _assemble(res.results), res



# revision 4
# speedup vs baseline: 1.6403x; 1.6403x over previous
"""Trainium2 Bass kernel for nn_DCConv3dKernelPolynomials.

Computes out[m,n,b,p] = sum_k coeff[m,n,k] * psi_k(position[b,p,:])
where psi_k are the 23 real hydrogen-like wavefunctions (n<=4, l<=2).

Key math: with u=x/r, v=y/r, w=z/r, the angular factors are pure
polynomials (st^|m| e^{i|m|phi} = (u+iv)^|m|, P_l^m = st^|m| * poly(ct)),
so the device only needs sqrt/reciprocal/exp plus polynomial arithmetic.

Sharding: batch b -> core b (8 cores, 4096 points each). Per core:
  poly bf16 [32, 4096] basis matrix (pointwise + PE transposes; k padded
  23->32 with zeros) replicated into the 4 SBUF partition quadrants via a
  host-side point permutation, so 4x row-tiled matmuls (tile_position via
  base_partition, K=32 per 32x128 sub-array) run 4 MMs concurrently.
  out [4096(mn), 4096(pts)] bf16 = coeffT.T @ poly streamed to HBM
  (converted to f32 on host; rel-err budget 2e-2 >> bf16's ~2e-3).
  PSUM: 3x [128,1024] 2-bank matmul tiles + 2 transpose banks = 8 banks.
"""

import math

import numpy as np

B = 8
PTS = 4096            # points per core
OUTC = INC = 64
MN = OUTC * INC       # 4096
NB = 23               # basis functions
KP = 32               # padded K per row-tile quadrant
NCORES = 8
PCHUNK = 32           # free-dim columns per partition in pointwise layout
NMT = MN // 128       # 32 mn tiles


def _combos():
    combos = []
    for n in range(1, 5):
        for k in range(3):
            for m in range(-3, 4):
                if abs(m) <= k and k < n:
                    combos.append((n, k, m))
    return combos


COMBOS = _combos()
assert len(COMBOS) == NB


def _basis_scales():
    """Per-basis constant c_k so that psi_k = c_k * Rb_dev(n,l) * A_dev(l,m).

    Rb_dev / A_dev are the *unnormalized* tile products computed on device:
      Rb_dev(1,0)=e1, Rb_dev(2,0)=(2-r)e2, Rb_dev(2,1)=r*e2,
      Rb_dev(3,0)=(2r-(2/9)r^2-3)e3   [= -L_2^1(2r/3)e3]
      Rb_dev(3,1)=((8/3)r-(4/9)r^2)e3, Rb_dev(3,2)=r^2*e3,
      Rb_dev(4,0)=(4-3r+r^2/2-r^3/48)e4,
      Rb_dev(4,1)=(5r-(5/4)r^2+r^3/16)e4, Rb_dev(4,2)=(6-r/2)r^2*e4
      A_dev: 1, w, u, v, 3w^2-1, wu, wv, u^2-v^2, uv
    """
    fourpi = 4.0 * math.pi
    K00 = math.sqrt(1.0 / fourpi)
    K10 = math.sqrt(3.0 / fourpi)
    K11 = math.sqrt(3.0 / (2.0 * fourpi))
    K20 = math.sqrt(5.0 / fourpi)
    K21 = math.sqrt(5.0 / (6.0 * fourpi))
    K22 = math.sqrt(5.0 / (24.0 * fourpi))
    s2 = math.sqrt(2.0)

    def norm_r(n, l):
        return math.sqrt(
            (2.0 / n) ** 3 * math.factorial(n - l - 1)
            / (2 * n * math.factorial(n + l))
        )

    f = {
        (1, 0): norm_r(1, 0),
        (2, 0): norm_r(2, 0),
        (2, 1): norm_r(2, 1),
        (3, 0): -norm_r(3, 0),
        (3, 1): norm_r(3, 1),
        (3, 2): norm_r(3, 2) * (4.0 / 9.0),
        (4, 0): norm_r(4, 0),
        (4, 1): norm_r(4, 1),
        (4, 2): norm_r(4, 2) * 0.25,
    }
    a = {
        (0, 0): K00,
        (1, 0): K10,
        (1, 1): -s2 * K11,
        (1, -1): -s2 * K11,
        (2, 0): 0.5 * K20,
        (2, 1): -3.0 * s2 * K21,
        (2, -1): -3.0 * s2 * K21,
        (2, 2): 3.0 * s2 * K22,
        (2, -2): 6.0 * s2 * K22,
    }
    return np.array(
        [f[(n, l)] * a[(l, m)] for (n, l, m) in COMBOS], dtype=np.float64
    )


def poly_host(position):
    """Numpy replica of the device basis recipe (for self-checking)."""
    pos = np.asarray(position, dtype=np.float32)
    x, y, z = pos[..., 0], pos[..., 1], pos[..., 2]
    r2 = x * x + y * y + z * z
    r = np.sqrt(r2)
    ir = 1.0 / r
    u, v, w = x * ir, y * ir, z * ir
    e1, e2, e3, e4 = np.exp(-r), np.exp(-r / 2), np.exp(-r / 3), np.exp(-r / 4)
    rr = r * r
    A = {
        (0, 0): np.ones_like(r),
        (1, 0): w, (1, 1): u, (1, -1): v,
        (2, 0): 3 * w * w - 1, (2, 1): w * u, (2, -1): w * v,
        (2, 2): u * u - v * v, (2, -2): u * v,
    }
    Rb = {
        (1, 0): e1,
        (2, 0): (2 - r) * e2,
        (2, 1): r * e2,
        (3, 0): (2 * r - (2.0 / 9.0) * rr - 3) * e3,
        (3, 1): ((8.0 / 3.0) * r - (4.0 / 9.0) * rr) * e3,
        (3, 2): rr * e3,
        (4, 0): (4 - 3 * r + rr / 2 - rr * r / 48) * e4,
        (4, 1): (5 * r - 1.25 * rr + rr * r / 16) * e4,
        (4, 2): (6 - r / 2) * rr * e4,
    }
    c = _basis_scales()
    return np.stack(
        [
            (c[k] * Rb[(n, l)] * A[(l, m)]).astype(np.float32)
            for k, (n, l, m) in enumerate(COMBOS)
        ],
        axis=-1,
    )


def _point_perm():
    """perm[p, c] = canonical point id held at pointwise slot (p, c).

    Chosen so the 4x row-tiled matmul outputs land contiguously:
    quadrant q = c%4, chunk-group cg = c//4, nt = cg//4, cgl = cg%4;
    point = 512*(2q + nt) + 128*cgl + p. PSUM tile (q) then covers
    canonical points [1024q, 1024q+1024) in order.
    """
    p = np.arange(128)[:, None]
    c = np.arange(PCHUNK)[None, :]
    q, cg = c % 4, c // 4
    nt, cgl = cg // 4, cg % 4
    return 512 * (2 * q + nt) + 128 * cgl + p


_PROGRAM = None


def _build_program():
    import concourse.bacc as bacc
    import concourse.tile as tile
    from concourse import mybir
    from concourse.bass import ts
    from concourse.masks import make_identity

    f32 = mybir.dt.float32
    bf16 = mybir.dt.bfloat16
    AF = mybir.ActivationFunctionType
    ALU = mybir.AluOpType

    nc = bacc.Bacc(trn_type="TRN2")
    pos_d = nc.dram_tensor("position", [128, 96], f32, kind="ExternalInput")
    coefft_d = nc.dram_tensor("coefft", [128, MN], bf16, kind="ExternalInput")
    out_d = nc.dram_tensor("out", [MN, PTS], bf16, kind="ExternalOutput")

    with tile.TileContext(nc) as tc:
        with (
            tc.tile_pool(name="const", bufs=1) as const,
            tc.tile_pool(name="pw", bufs=1) as pw,
            tc.tile_pool(name="stage", bufs=3) as stage_pool,
            tc.tile_pool(name="psum_tr", bufs=2, space="PSUM") as psum_tr,
            tc.tile_pool(name="psum_mm", bufs=3, space="PSUM") as psum_mm,
        ):
            # inputs first: xyz gates the whole pointwise phase.
            # SWDGE (gpsimd) sprays across all 16 SDMA engines.
            xyz = const.tile([128, 96], f32, tag="xyz", name="xyz")
            nc.gpsimd.dma_start(out=xyz[:], in_=pos_d[:, :])
            coefft = const.tile([128, MN], bf16, tag="coefft", name="coefft_sb")
            nc.gpsimd.dma_start(out=coefft[:], in_=coefft_d[:, :])

            ident = const.tile([128, 128], bf16, tag="ident", name="ident")
            make_identity(nc, ident[:])

            xyz3 = xyz[:].rearrange("p (c t) -> p c t", t=3)
            x, y, z = xyz3[:, :, 0], xyz3[:, :, 1], xyz3[:, :, 2]

            def T(tag):
                return pw.tile([128, PCHUNK], f32, tag=tag, name=tag)[:]

            def bcast3(ap2d, n):
                import concourse.bass as bass
                return bass.AP(
                    tensor=ap2d.tensor,
                    offset=ap2d.offset,
                    ap=[ap2d.ap[0], [0, n], ap2d.ap[1]],
                )

            # basis values poly_s[:, c, k] bf16, k padded 23->32 with zeros
            poly_s = const.tile([128, PCHUNK, KP], bf16, tag="poly_s", name="poly_s")
            nc.gpsimd.memset(poly_s[:, :, NB:KP], 0.0)

            # ---- pointwise: r, 1/r via exp(+-0.5*ln(r2)) -- one ACT table set
            r2, r, ir, rr = (T(t) for t in "r2 r ir rr".split())
            lnr2 = T("lnr2")
            nc.vector.tensor_mul(r2, x, x)
            tA, tB = T("tA"), T("tB")
            nc.vector.tensor_mul(tA, y, y)
            nc.vector.tensor_add(r2, r2, tA)
            nc.vector.tensor_mul(tB, z, z)
            nc.vector.tensor_add(r2, r2, tB)
            nc.scalar.activation(lnr2, r2, AF.Ln)
            nc.scalar.activation(r, lnr2, AF.Exp, scale=0.5)
            nc.scalar.activation(ir, lnr2, AF.Exp, scale=-0.5)

            # vwu[:, s, :] = (v, w, u); ang5[:, s, :] = (uv, wv, 3w^2-1, wu, u^2-v^2)
            vwu = pw.tile([128, 3, PCHUNK], f32, tag="vwu", name="vwu")[:]
            ang5 = pw.tile([128, 5, PCHUNK], f32, tag="ang5", name="ang5")[:]
            v, w, u = vwu[:, 0, :], vwu[:, 1, :], vwu[:, 2, :]
            uv, wv, a20, wu, a22 = (ang5[:, i, :] for i in range(5))
            nc.vector.tensor_mul(v, y, ir)
            nc.vector.tensor_mul(w, z, ir)
            nc.vector.tensor_mul(u, x, ir)
            nc.vector.tensor_mul(rr, r, r)

            e2, e3, e4 = T("e2"), T("e3"), T("e4")
            nc.scalar.activation(e2, r, AF.Exp, scale=-0.5)
            nc.scalar.activation(e3, r, AF.Exp, scale=-1.0 / 3.0)
            nc.scalar.activation(e4, r, AF.Exp, scale=-0.25)

            uu, vv = T("uu"), T("vv")
            nc.vector.tensor_mul(a20, w, w)
            nc.vector.tensor_scalar(a20, a20, 3.0, -1.0, ALU.mult, ALU.add)
            nc.vector.tensor_mul(uu, u, u)
            nc.vector.tensor_mul(vv, v, v)
            nc.vector.tensor_sub(a22, uu, vv)
            nc.vector.tensor_mul(uv, u, v)
            nc.vector.tensor_mul(wu, w, u)
            nc.vector.tensor_mul(wv, w, v)

            slot = [poly_s[:, :, k] for k in range(NB)]
            poly_kc = poly_s[:].rearrange("p c k -> p k c")

            nc.scalar.activation(slot[0], r, AF.Exp, scale=-1.0)
            t20 = T("t20")
            nc.vector.tensor_scalar(t20, r, -1.0, 2.0, ALU.mult, ALU.add)
            nc.vector.tensor_mul(slot[1], t20, e2)
            rb21 = T("rb21")
            nc.vector.tensor_mul(rb21, r, e2)
            nc.vector.tensor_mul(poly_kc[:, 2:5, :], bcast3(rb21, 3), vwu)
            p30 = T("p30")
            nc.vector.tensor_scalar(p30, rr, 2.0 / 9.0, 3.0, ALU.mult, ALU.add)
            nc.vector.scalar_tensor_tensor(
                p30, r, 2.0, p30, ALU.mult, ALU.subtract
            )
            nc.vector.tensor_mul(slot[5], p30, e3)
            rb31 = T("rb31")
            nc.vector.tensor_scalar(
                rb31, r, -4.0 / 9.0, 8.0 / 3.0, ALU.mult, ALU.add
            )
            nc.vector.tensor_mul(rb31, rb31, r)
            nc.vector.tensor_mul(rb31, rb31, e3)
            nc.vector.tensor_mul(poly_kc[:, 6:9, :], bcast3(rb31, 3), vwu)
            rb32 = T("rb32")
            nc.vector.tensor_mul(rb32, rr, e3)
            nc.vector.tensor_mul(poly_kc[:, 9:14, :], bcast3(rb32, 5), ang5)
            p40, p40b = T("p40"), T("p40b")
            nc.vector.tensor_scalar(p40, r, -1.0 / 48.0, 0.5, ALU.mult, ALU.add)
            nc.vector.tensor_mul(p40, p40, rr)
            nc.vector.tensor_scalar(p40b, r, -3.0, 4.0, ALU.mult, ALU.add)
            nc.vector.tensor_add(p40, p40, p40b)
            nc.vector.tensor_mul(slot[14], p40, e4)
            rb41 = T("rb41")
            nc.vector.tensor_scalar(rb41, r, 1.0 / 16.0, -1.25, ALU.mult, ALU.add)
            nc.vector.tensor_mul(rb41, rb41, r)
            nc.vector.tensor_scalar(rb41, rb41, 5.0, None, ALU.add)
            nc.vector.tensor_mul(rb41, rb41, r)
            nc.vector.tensor_mul(rb41, rb41, e4)
            nc.vector.tensor_mul(poly_kc[:, 15:18, :], bcast3(rb41, 3), vwu)
            rb42 = T("rb42")
            nc.vector.tensor_scalar(rb42, r, -0.5, 6.0, ALU.mult, ALU.add)
            nc.vector.tensor_mul(rb42, rb42, rr)
            nc.vector.tensor_mul(rb42, rb42, e4)
            nc.vector.tensor_mul(poly_kc[:, 18:23, :], bcast3(rb42, 5), ang5)

            # ---- transpose to poly4 [128, 1024]: quadrant row 32q+k holds
            # psi_k of points (p, c=4*cg+q) at col 128*cg+p ----
            poly4 = const.tile([128, 1024], bf16, tag="poly4", name="poly4")
            poly_flat = poly_s[:].rearrange("p c k -> p (c k)")
            for cg in range(8):
                pst = psum_tr.tile([128, 128], bf16, tag="pst", name="pst")
                nc.tensor.transpose(pst[:], poly_flat[:, ts(cg, 128)], ident[:])
                nc.vector.tensor_copy(poly4[:, ts(cg, 128)], pst[:])

            # ---- 4x row-tiled matmuls + copies + output DMA ----
            # per mt: for q in 0..3 one [128,1024] 2-bank psum (nt=0,1 halves);
            # tile_position=(32q, 0) derives from lhsT/rhs base partition.
            def do_mt(mt, sl):
                for q in range(4):
                    ps = psum_mm.tile([128, 1024], f32, tag="mmps", name="mmps")
                    for nt in range(2):
                        nc.tensor.matmul(
                            ps[:, ts(nt, 512)],
                            lhsT=coefft[ts(q, 32), ts(mt, 128)],
                            rhs=poly4[ts(q, 32), ts(nt, 512)],
                            start=True,
                            stop=True,
                            tile_position=(32 * q, 0),
                        )
                    if q % 2 == 0:
                        nc.vector.tensor_copy(sl(ts(q, 1024)), ps[:])
                    else:
                        nc.scalar.copy(sl(ts(q, 1024)), ps[:])

            # stage sizes: 1,1 then 14x2 then 1,1 (fast start / short tail)
            sizes = [1, 1] + [2] * 14 + [1, 1]
            mt0 = 0
            for si, sz in enumerate(sizes):
                stage = stage_pool.tile(
                    [128, sz, PTS], bf16, tag=f"stage{sz}", name="stage"
                )
                for s2 in range(sz):
                    do_mt(mt0 + s2, lambda s, _s2=s2: stage[:, _s2, s])
                dma_eng = nc.sync if si % 2 == 0 else nc.scalar
                dest = out_d[mt0 * 128:(mt0 + sz) * 128, :].rearrange(
                    "(s p) j -> p s j", p=128
                )
                dma_eng.dma_start(out=dest, in_=stage[:, :, :])
                mt0 += sz
            assert mt0 == NMT

    nc.finalize()
    return nc


def _get_program():
    global _PROGRAM
    if _PROGRAM is None:
        _PROGRAM = _build_program()
    return _PROGRAM


def _prep_inputs(position, coefficients):
    import ml_dtypes

    pos = np.ascontiguousarray(np.asarray(position, dtype=np.float32))
    coeff = np.asarray(coefficients, dtype=np.float32)
    assert pos.shape == (B, PTS, 3) and coeff.shape == (OUTC, INC, NB)
    c = _basis_scales().astype(np.float32)
    C = (coeff * c).reshape(MN, NB).T.astype(np.float32)  # [23, 4096]
    coefft = np.zeros((128, MN), dtype=ml_dtypes.bfloat16)
    for q in range(4):
        coefft[32 * q:32 * q + NB, :] = C.astype(ml_dtypes.bfloat16)
    perm = _point_perm()  # [128, 32] -> canonical point ids
    return [
        {
            "position": np.ascontiguousarray(
                pos[b][perm].reshape(128, 96)
            ),
            "coefft": coefft,
        }
        for b in range(B)
    ]


def _assemble(results):
    return np.stack(
        [
            np.asarray(r["out"]).astype(np.float32).reshape(OUTC, INC, PTS)
            for r in results
        ],
        axis=2,
    )


def kernel(position, coefficients):
    from concourse import bass_utils

    nc = _get_program()
    in_maps = _prep_inputs(position, coefficients)
    res = bass_utils.run_bass_kernel_spmd(nc, in_maps, core_ids=list(range(NCORES)))
    return _assemble(res.results)


def kernel_traced(position, coefficients, trace_cores=None):
    """Like kernel() but captures an NTFF trace; returns (out, results)."""
    from concourse import bass_utils

    nc = _get_program()
    in_maps = _prep_inputs(position, coefficients)
    res = bass_utils.run_bass_kernel_spmd(
        nc,
        in_maps,
        core_ids=list(range(NCORES)),
        trace=True,
        trace_cores=trace_cores,
    )
    return _assemble(res.results), res


# revision 6
# speedup vs baseline: 1.7726x; 1.0807x over previous
"""Trainium2 Bass kernel for nn_DCConv3dKernelPolynomials.

Computes out[m,n,b,p] = sum_k coeff[m,n,k] * psi_k(position[b,p,:])
where psi_k are the 23 real hydrogen-like wavefunctions (n<=4, l<=2).

Key math: with u=x/r, v=y/r, w=z/r, the angular factors are pure
polynomials (st^|m| e^{i|m|phi} = (u+iv)^|m|, P_l^m = st^|m| * poly(ct)),
so the device only needs sqrt/reciprocal/exp plus polynomial arithmetic.

Sharding: batch b -> core b (8 cores, 4096 points each). Per core:
  poly bf16 [32, 4096] basis matrix (pointwise + PE transposes; k padded
  23->32 with zeros) replicated into the 4 SBUF partition quadrants via a
  host-side point permutation, so 4x row-tiled matmuls (tile_position via
  base_partition, K=32 per 32x128 sub-array) run 4 MMs concurrently.
  out [4096(mn), 4096(pts)] bf16 = coeffT.T @ poly streamed to HBM
  (converted to f32 on host; rel-err budget 2e-2 >> bf16's ~2e-3).
  PSUM: 3x [128,1024] 2-bank matmul tiles + 2 transpose banks = 8 banks.
"""

import math

import numpy as np

B = 8
PTS = 4096            # points per core
OUTC = INC = 64
MN = OUTC * INC       # 4096
NB = 23               # basis functions
KP = 32               # padded K per row-tile quadrant
NCORES = 8
PCHUNK = 32           # free-dim columns per partition in pointwise layout
NMT = MN // 128       # 32 mn tiles


def _combos():
    combos = []
    for n in range(1, 5):
        for k in range(3):
            for m in range(-3, 4):
                if abs(m) <= k and k < n:
                    combos.append((n, k, m))
    return combos


COMBOS = _combos()
assert len(COMBOS) == NB


def _basis_scales():
    """Per-basis constant c_k so that psi_k = c_k * Rb_dev(n,l) * A_dev(l,m).

    Rb_dev / A_dev are the *unnormalized* tile products computed on device:
      Rb_dev(1,0)=e1, Rb_dev(2,0)=(2-r)e2, Rb_dev(2,1)=r*e2,
      Rb_dev(3,0)=(2r-(2/9)r^2-3)e3   [= -L_2^1(2r/3)e3]
      Rb_dev(3,1)=((8/3)r-(4/9)r^2)e3, Rb_dev(3,2)=r^2*e3,
      Rb_dev(4,0)=(4-3r+r^2/2-r^3/48)e4,
      Rb_dev(4,1)=(5r-(5/4)r^2+r^3/16)e4, Rb_dev(4,2)=(6-r/2)r^2*e4
      A_dev: 1, w, u, v, 3w^2-1, wu, wv, u^2-v^2, uv
    """
    fourpi = 4.0 * math.pi
    K00 = math.sqrt(1.0 / fourpi)
    K10 = math.sqrt(3.0 / fourpi)
    K11 = math.sqrt(3.0 / (2.0 * fourpi))
    K20 = math.sqrt(5.0 / fourpi)
    K21 = math.sqrt(5.0 / (6.0 * fourpi))
    K22 = math.sqrt(5.0 / (24.0 * fourpi))
    s2 = math.sqrt(2.0)

    def norm_r(n, l):
        return math.sqrt(
            (2.0 / n) ** 3 * math.factorial(n - l - 1)
            / (2 * n * math.factorial(n + l))
        )

    f = {
        (1, 0): norm_r(1, 0),
        (2, 0): norm_r(2, 0),
        (2, 1): norm_r(2, 1),
        (3, 0): -norm_r(3, 0),
        (3, 1): norm_r(3, 1),
        (3, 2): norm_r(3, 2) * (4.0 / 9.0),
        (4, 0): norm_r(4, 0),
        (4, 1): norm_r(4, 1),
        (4, 2): norm_r(4, 2) * 0.25,
    }
    a = {
        (0, 0): K00,
        (1, 0): K10,
        (1, 1): -s2 * K11,
        (1, -1): -s2 * K11,
        (2, 0): 0.5 * K20,
        (2, 1): -3.0 * s2 * K21,
        (2, -1): -3.0 * s2 * K21,
        (2, 2): 3.0 * s2 * K22,
        (2, -2): 6.0 * s2 * K22,
    }
    return np.array(
        [f[(n, l)] * a[(l, m)] for (n, l, m) in COMBOS], dtype=np.float64
    )


def poly_host(position):
    """Numpy replica of the device basis recipe (for self-checking)."""
    pos = np.asarray(position, dtype=np.float32)
    x, y, z = pos[..., 0], pos[..., 1], pos[..., 2]
    r2 = x * x + y * y + z * z
    r = np.sqrt(r2)
    ir = 1.0 / r
    u, v, w = x * ir, y * ir, z * ir
    e1, e2, e3, e4 = np.exp(-r), np.exp(-r / 2), np.exp(-r / 3), np.exp(-r / 4)
    rr = r * r
    A = {
        (0, 0): np.ones_like(r),
        (1, 0): w, (1, 1): u, (1, -1): v,
        (2, 0): 3 * w * w - 1, (2, 1): w * u, (2, -1): w * v,
        (2, 2): u * u - v * v, (2, -2): u * v,
    }
    Rb = {
        (1, 0): e1,
        (2, 0): (2 - r) * e2,
        (2, 1): r * e2,
        (3, 0): (2 * r - (2.0 / 9.0) * rr - 3) * e3,
        (3, 1): ((8.0 / 3.0) * r - (4.0 / 9.0) * rr) * e3,
        (3, 2): rr * e3,
        (4, 0): (4 - 3 * r + rr / 2 - rr * r / 48) * e4,
        (4, 1): (5 * r - 1.25 * rr + rr * r / 16) * e4,
        (4, 2): (6 - r / 2) * rr * e4,
    }
    c = _basis_scales()
    return np.stack(
        [
            (c[k] * Rb[(n, l)] * A[(l, m)]).astype(np.float32)
            for k, (n, l, m) in enumerate(COMBOS)
        ],
        axis=-1,
    )


def _point_perm():
    """perm[p, c] = canonical point id held at pointwise slot (p, c).

    Chosen so the 4x row-tiled matmul outputs land contiguously:
    quadrant q = c%4, chunk-group cg = c//4, nt = cg//4, cgl = cg%4;
    point = 512*(2q + nt) + 128*cgl + p. PSUM tile (q) then covers
    canonical points [1024q, 1024q+1024) in order.
    """
    p = np.arange(128)[:, None]
    c = np.arange(PCHUNK)[None, :]
    q, cg = c % 4, c // 4
    nt, cgl = cg // 4, cg % 4
    return 512 * (2 * q + nt) + 128 * cgl + p


_PROGRAM = None


def _build_program():
    import concourse.bacc as bacc
    import concourse.tile as tile
    from concourse import mybir
    from concourse.bass import ts
    from concourse.masks import make_identity

    f32 = mybir.dt.float32
    bf16 = mybir.dt.bfloat16
    AF = mybir.ActivationFunctionType
    ALU = mybir.AluOpType

    nc = bacc.Bacc(trn_type="TRN2")
    pos_d = nc.dram_tensor("position", [128, 96], f32, kind="ExternalInput")
    coefft_d = nc.dram_tensor("coefft", [128, MN], bf16, kind="ExternalInput")
    out_d = nc.dram_tensor("out", [MN, PTS], bf16, kind="ExternalOutput")

    with tile.TileContext(nc) as tc:
        with (
            tc.tile_pool(name="const", bufs=1) as const,
            tc.tile_pool(name="pw", bufs=1) as pw,
            tc.tile_pool(name="stage", bufs=3) as stage_pool,
            tc.tile_pool(name="psum_mm", bufs=4, space="PSUM") as psum_mm,
        ):
            # inputs first: xyz gates the whole pointwise phase.
            # SWDGE (gpsimd) sprays across all 16 SDMA engines.
            xyz = const.tile([128, 96], f32, tag="xyz", name="xyz")
            nc.gpsimd.dma_start(out=xyz[:], in_=pos_d[:, :])
            coefft = const.tile([128, MN], bf16, tag="coefft", name="coefft_sb")
            nc.gpsimd.dma_start(out=coefft[:], in_=coefft_d[:, :])

            ident = const.tile([128, 128], bf16, tag="ident", name="ident")
            make_identity(nc, ident[:])

            xyz3 = xyz[:].rearrange("p (c t) -> p c t", t=3)
            x, y, z = xyz3[:, :, 0], xyz3[:, :, 1], xyz3[:, :, 2]

            def T(tag):
                return pw.tile([128, PCHUNK], f32, tag=tag, name=tag)[:]

            def bcast3(ap2d, n):
                import concourse.bass as bass
                return bass.AP(
                    tensor=ap2d.tensor,
                    offset=ap2d.offset,
                    ap=[ap2d.ap[0], [0, n], ap2d.ap[1]],
                )

            # basis values poly_s[:, c, k] bf16, k padded 23->32 with zeros
            poly_s = const.tile([128, PCHUNK, KP], bf16, tag="poly_s", name="poly_s")
            nc.gpsimd.memset(poly_s[:, :, NB:KP], 0.0)

            # ---- pointwise: r, 1/r via exp(+-0.5*ln(r2)) -- one ACT table set
            r2, r, ir, rr = (T(t) for t in "r2 r ir rr".split())
            lnr2 = T("lnr2")
            nc.vector.tensor_mul(r2, x, x)
            tA, tB = T("tA"), T("tB")
            nc.vector.tensor_mul(tA, y, y)
            nc.vector.tensor_add(r2, r2, tA)
            nc.vector.tensor_mul(tB, z, z)
            nc.vector.tensor_add(r2, r2, tB)
            nc.scalar.activation(lnr2, r2, AF.Ln)
            nc.scalar.activation(r, lnr2, AF.Exp, scale=0.5)
            nc.scalar.activation(ir, lnr2, AF.Exp, scale=-0.5)

            # vwu[:, s, :] = (v, w, u); ang5[:, s, :] = (uv, wv, 3w^2-1, wu, u^2-v^2)
            vwu = pw.tile([128, 3, PCHUNK], f32, tag="vwu", name="vwu")[:]
            ang5 = pw.tile([128, 5, PCHUNK], f32, tag="ang5", name="ang5")[:]
            v, w, u = vwu[:, 0, :], vwu[:, 1, :], vwu[:, 2, :]
            uv, wv, a20, wu, a22 = (ang5[:, i, :] for i in range(5))
            nc.vector.tensor_mul(v, y, ir)
            nc.vector.tensor_mul(w, z, ir)
            nc.vector.tensor_mul(u, x, ir)
            nc.vector.tensor_mul(rr, r, r)

            e2, e3, e4 = T("e2"), T("e3"), T("e4")
            nc.scalar.activation(e2, r, AF.Exp, scale=-0.5)
            nc.scalar.activation(e3, r, AF.Exp, scale=-1.0 / 3.0)
            nc.scalar.activation(e4, r, AF.Exp, scale=-0.25)

            uu, vv = T("uu"), T("vv")
            nc.vector.tensor_mul(a20, w, w)
            nc.vector.tensor_scalar(a20, a20, 3.0, -1.0, ALU.mult, ALU.add)
            nc.vector.tensor_mul(uu, u, u)
            nc.vector.tensor_mul(vv, v, v)
            nc.vector.tensor_sub(a22, uu, vv)
            nc.vector.tensor_mul(uv, u, v)
            nc.vector.tensor_mul(wu, w, u)
            nc.vector.tensor_mul(wv, w, v)

            slot = [poly_s[:, :, k] for k in range(NB)]
            poly_kc = poly_s[:].rearrange("p c k -> p k c")

            nc.scalar.activation(slot[0], r, AF.Exp, scale=-1.0)
            t20 = T("t20")
            nc.vector.tensor_scalar(t20, r, -1.0, 2.0, ALU.mult, ALU.add)
            nc.vector.tensor_mul(slot[1], t20, e2)
            rb21 = T("rb21")
            nc.vector.tensor_mul(rb21, r, e2)
            nc.vector.tensor_mul(poly_kc[:, 2:5, :], bcast3(rb21, 3), vwu)
            p30 = T("p30")
            nc.vector.tensor_scalar(p30, rr, 2.0 / 9.0, 3.0, ALU.mult, ALU.add)
            nc.vector.scalar_tensor_tensor(
                p30, r, 2.0, p30, ALU.mult, ALU.subtract
            )
            nc.vector.tensor_mul(slot[5], p30, e3)
            rb31 = T("rb31")
            nc.vector.tensor_scalar(
                rb31, r, -4.0 / 9.0, 8.0 / 3.0, ALU.mult, ALU.add
            )
            nc.vector.tensor_mul(rb31, rb31, r)
            nc.vector.tensor_mul(rb31, rb31, e3)
            nc.vector.tensor_mul(poly_kc[:, 6:9, :], bcast3(rb31, 3), vwu)
            rb32 = T("rb32")
            nc.vector.tensor_mul(rb32, rr, e3)
            nc.vector.tensor_mul(poly_kc[:, 9:14, :], bcast3(rb32, 5), ang5)
            p40, p40b = T("p40"), T("p40b")
            nc.vector.tensor_scalar(p40, r, -1.0 / 48.0, 0.5, ALU.mult, ALU.add)
            nc.vector.tensor_mul(p40, p40, rr)
            nc.vector.tensor_scalar(p40b, r, -3.0, 4.0, ALU.mult, ALU.add)
            nc.vector.tensor_add(p40, p40, p40b)
            nc.vector.tensor_mul(slot[14], p40, e4)
            rb41 = T("rb41")
            nc.vector.tensor_scalar(rb41, r, 1.0 / 16.0, -1.25, ALU.mult, ALU.add)
            nc.vector.tensor_mul(rb41, rb41, r)
            nc.vector.tensor_scalar(rb41, rb41, 5.0, None, ALU.add)
            nc.vector.tensor_mul(rb41, rb41, r)
            nc.vector.tensor_mul(rb41, rb41, e4)
            nc.vector.tensor_mul(poly_kc[:, 15:18, :], bcast3(rb41, 3), vwu)
            rb42 = T("rb42")
            nc.vector.tensor_scalar(rb42, r, -0.5, 6.0, ALU.mult, ALU.add)
            nc.vector.tensor_mul(rb42, rb42, rr)
            nc.vector.tensor_mul(rb42, rb42, e4)
            nc.vector.tensor_mul(poly_kc[:, 18:23, :], bcast3(rb42, 5), ang5)

            # ---- transpose to poly4 [128, 1024]: quadrant row 32q+k holds
            # psi_k of points (p, c=4*cg+q) at col 128*cg+p ----
            poly4 = const.tile([128, 1024], bf16, tag="poly4", name="poly4")
            poly_flat = poly_s[:].rearrange("p c k -> p (c k)")
            for cg in range(8):
                pst = psum_mm.tile([128, 128], bf16, tag="mmps", name="pst")
                nc.tensor.transpose(pst[:], poly_flat[:, ts(cg, 128)], ident[:])
                nc.vector.tensor_copy(poly4[:, ts(cg, 128)], pst[:])

            # ---- 4x row-tiled matmuls + copies + output DMA ----
            # per mt: for q in 0..3 one [128,1024] 2-bank psum (nt=0,1 halves);
            # tile_position=(32q, 0) derives from lhsT/rhs base partition.
            def do_mt(mt, sl):
                for q in range(4):
                    ps = psum_mm.tile([128, 1024], f32, tag="mmps", name="mmps")
                    for nt in range(2):
                        nc.tensor.matmul(
                            ps[:, ts(nt, 512)],
                            lhsT=coefft[ts(q, 32), ts(mt, 128)],
                            rhs=poly4[ts(q, 32), ts(nt, 512)],
                            start=True,
                            stop=True,
                            tile_position=(32 * q, 0),
                        )
                    if q % 2 == 0:
                        nc.vector.tensor_copy(sl(ts(q, 1024)), ps[:])
                    else:
                        nc.scalar.copy(sl(ts(q, 1024)), ps[:])

            # stage sizes: 1,1 then 14x2 then 1,1 (fast start / short tail)
            sizes = [1, 1] + [2] * 14 + [1, 1]
            mt0 = 0
            for si, sz in enumerate(sizes):
                stage = stage_pool.tile(
                    [128, sz, PTS], bf16, tag=f"stage{sz}", name="stage"
                )
                for s2 in range(sz):
                    do_mt(mt0 + s2, lambda s, _s2=s2: stage[:, _s2, s])
                dma_eng = nc.sync if si % 2 == 0 else nc.scalar
                dest = out_d[mt0 * 128:(mt0 + sz) * 128, :].rearrange(
                    "(s p) j -> p s j", p=128
                )
                dma_eng.dma_start(out=dest, in_=stage[:, :, :])
                mt0 += sz
            assert mt0 == NMT

    nc.finalize()
    return nc


def _get_program():
    global _PROGRAM
    if _PROGRAM is None:
        _PROGRAM = _build_program()
    return _PROGRAM


def _prep_inputs(position, coefficients):
    import ml_dtypes

    pos = np.ascontiguousarray(np.asarray(position, dtype=np.float32))
    coeff = np.asarray(coefficients, dtype=np.float32)
    assert pos.shape == (B, PTS, 3) and coeff.shape == (OUTC, INC, NB)
    c = _basis_scales().astype(np.float32)
    C = (coeff * c).reshape(MN, NB).T.astype(np.float32)  # [23, 4096]
    coefft = np.zeros((128, MN), dtype=ml_dtypes.bfloat16)
    for q in range(4):
        coefft[32 * q:32 * q + NB, :] = C.astype(ml_dtypes.bfloat16)
    perm = _point_perm()  # [128, 32] -> canonical point ids
    return [
        {
            "position": np.ascontiguousarray(
                pos[b][perm].reshape(128, 96)
            ),
            "coefft": coefft,
        }
        for b in range(B)
    ]


def _assemble(results):
    return np.stack(
        [
            np.asarray(r["out"]).astype(np.float32).reshape(OUTC, INC, PTS)
            for r in results
        ],
        axis=2,
    )


def kernel(position, coefficients):
    from concourse import bass_utils

    nc = _get_program()
    in_maps = _prep_inputs(position, coefficients)
    res = bass_utils.run_bass_kernel_spmd(nc, in_maps, core_ids=list(range(NCORES)))
    return _assemble(res.results)


def kernel_traced(position, coefficients, trace_cores=None):
    """Like kernel() but captures an NTFF trace; returns (out, results)."""
    from concourse import bass_utils

    nc = _get_program()
    in_maps = _prep_inputs(position, coefficients)
    res = bass_utils.run_bass_kernel_spmd(
        nc,
        in_maps,
        core_ids=list(range(NCORES)),
        trace=True,
        trace_cores=trace_cores,
    )
    return _assemble(res.results), res
